# revision 6
# baseline (speedup 1.0000x reference)
"""MLP (additive) attention kernel for Trainium2, 8 NeuronCores.

scores[b,q,k] = sum_u v_u * tanh(qp[b,q,u] + kp[b,k,u]);  qp = query@W_q.T, kp = key@W_k.T
out = softmax(mask(scores)) @ value

Strategy: only k-columns below valid_len[b] are computed (masked columns
contribute exactly 0 to the softmax).  The valid region is split into
128-row k-tiles; each of the 8 cores processes NT = ceil(T/8) independent
(batch, k-tile) units and emits unnormalized partial outputs
O_t = exp(scores_t) @ V_t and row sums s_t; the host sums partials per
batch and divides.  Exact math (no approximations): tanh on ScalarE,
broadcast-add on VectorE/GpSimd, contractions on TensorE, fp32 throughout.
"""

import sys
import numpy as np
from contextlib import ExitStack

sys.path.insert(0, "/opt/trn_rl_repo")

import concourse.bass as bass
import concourse.tile as tile
from concourse import mybir, bacc, masks
from concourse._compat import get_trn_type
from concourse.bass_utils import run_bass_kernel_spmd

F32 = mybir.dt.float32
AF = mybir.ActivationFunctionType

B, LQ, LK = 4, 128, 1024
DQ, DK, DV, U = 512, 512, 512, 256
NCORES = 8
KT = 128            # k-tile rows
QCH = 16            # q rows per add/tanh chunk
MASK_VAL = -1e6

_cache = {}
TRACE = False
LAST_RESULT = None


def _build(nt: int):
    """Build + compile the SPMD program for nt units per core."""
    nc = bacc.Bacc(get_trn_type() or "TRN2", target_bir_lowering=False,
                   debug=False, enable_asserts=True, num_devices=NCORES)

    qt_d = nc.dram_tensor("qt", [nt, DQ, LQ], F32, kind="ExternalInput").ap()
    kt_d = nc.dram_tensor("kt", [nt, DK, KT], F32, kind="ExternalInput").ap()
    vt_d = nc.dram_tensor("vt", [nt, KT, DV], F32, kind="ExternalInput").ap()
    mk_d = nc.dram_tensor("mk", [nt, 1, KT], F32, kind="ExternalInput").ap()
    wq_d = nc.dram_tensor("wq", [DQ, U], F32, kind="ExternalInput").ap()
    wk_d = nc.dram_tensor("wk", [DK, U], F32, kind="ExternalInput").ap()
    vu_d = nc.dram_tensor("vu", [128, 2], F32, kind="ExternalInput").ap()
    o_d = nc.dram_tensor("o_part", [nt, LQ, DV], F32, kind="ExternalOutput").ap()
    s_d = nc.dram_tensor("s_part", [nt, 1, LQ], F32, kind="ExternalOutput").ap()

    with tile.TileContext(nc, trace_sim=False) as tc, ExitStack() as ctx:
        const = ctx.enter_context(tc.tile_pool(name="const", bufs=1))
        io = ctx.enter_context(tc.tile_pool(name="io", bufs=2))
        work = ctx.enter_context(tc.tile_pool(name="work", bufs=2))
        small = ctx.enter_context(tc.tile_pool(name="small", bufs=2))
        pp = ctx.enter_context(tc.tile_pool(name="pp", bufs=2, space="PSUM"))
        psc = ctx.enter_context(tc.tile_pool(name="psc", bufs=2, space="PSUM"))
        pet = ctx.enter_context(tc.tile_pool(name="pet", bufs=1, space="PSUM"))
        pout = ctx.enter_context(tc.tile_pool(name="pout", bufs=2, space="PSUM"))

        # constants
        wq_sb = const.tile([128, 4 * U], F32, tag="wq")   # (dchunk, u)
        wk_sb = const.tile([128, 4 * U], F32, tag="wk")
        vu_sb = const.tile([128, 2], F32, tag="vu")
        onek_sb = const.tile([128, 1], F32, tag="onek")
        ones_sb = const.tile([1, LQ], F32, tag="ones")
        for d in range(4):
            nc.sync.dma_start(wq_sb[:, d * U:(d + 1) * U], wq_d[bass.ts(d, 128), :])
            nc.sync.dma_start(wk_sb[:, d * U:(d + 1) * U], wk_d[bass.ts(d, 128), :])
        nc.sync.dma_start(vu_sb[:], vu_d[:])
        nc.vector.memset(ones_sb[:], 1.0)
        nc.vector.memset(onek_sb[:], 1.0)

        for t in range(nt):
            # ---- load unit inputs (HWDGE; consumers are PE) ----
            qt_sb = io.tile([128, 4 * LQ], F32, tag="qt")   # (dchunk, q)
            kt_sb = io.tile([128, 4 * KT], F32, tag="kt")
            vt_sb = io.tile([KT, DV], F32, tag="vt")
            mk_sb = io.tile([1, KT], F32, tag="mk")
            for d in range(4):
                nc.sync.dma_start(qt_sb[:, bass.ts(d, LQ)], qt_d[t, bass.ts(d, 128), :])
                nc.sync.dma_start(kt_sb[:, bass.ts(d, KT)], kt_d[t, bass.ts(d, 128), :])
            nc.sync.dma_start(vt_sb[:], vt_d[t])
            nc.sync.dma_start(mk_sb[:], mk_d[t])

            # ---- projections (transposed): proj_ps = [qp | kp], cols (uhi,q)/(uhi,k) ----
            proj = pp.tile([128, 2 * U], F32, tag="proj")  # 0:256 qp, 256:512 kp
            for uh in range(2):
                for d in range(4):
                    nc.tensor.matmul(
                        proj[:, bass.ts(uh, 128)],
                        wq_sb[:, d * U + uh * 128:d * U + (uh + 1) * 128],
                        qt_sb[:, bass.ts(d, LQ)],
                        start=(d == 0), stop=(d == 3))
            for uh in range(2):
                for d in range(4):
                    nc.tensor.matmul(
                        proj[:, 256 + uh * 128:256 + (uh + 1) * 128],
                        wk_sb[:, d * U + uh * 128:d * U + (uh + 1) * 128],
                        kt_sb[:, bass.ts(d, KT)],
                        start=(d == 0), stop=(d == 3))

            proj_sb = work.tile([128, 2 * U], F32, tag="projsb")
            nc.vector.tensor_copy(proj_sb[:], proj[:])

            scoresT = psc.tile([KT, LQ], F32, tag="scores")
            # mask as rank-1 first write: scoresT = mask^T(k) x ones(q)
            nc.tensor.matmul(scoresT[:], mk_sb[:], ones_sb[:], start=True, stop=False)

            for qc in range(LQ // QCH):
                s_t = work.tile([128, 2, QCH, KT], F32, tag="s")
                th_t = work.tile([128, 2, QCH, KT], F32, tag="th")
                for uh in range(2):
                    qp_ap = (proj_sb[:, uh * 128 + qc * QCH: uh * 128 + (qc + 1) * QCH]
                             .unsqueeze(2).broadcast_to([128, QCH, KT]))
                    kp_ap = (proj_sb[:, 256 + uh * 128:256 + (uh + 1) * 128]
                             .unsqueeze(1).broadcast_to([128, QCH, KT]))
                    eng = nc.vector if (qc + uh) % 3 else nc.gpsimd
                    eng.tensor_add(s_t[:, uh], qp_ap, kp_ap)
                nc.scalar.activation(
                    th_t[:].rearrange("p h q k -> p (h q k)"),
                    s_t[:].rearrange("p h q k -> p (h q k)"), AF.Tanh)
                # v-dot: one N=1 column matmul per (uh, q) accumulating
                # scoresT[:, qg] += TH[:, uh, qg].T @ v_half
                for uh in range(2):
                    for qq in range(QCH):
                        qg = qc * QCH + qq
                        last = (qc == LQ // QCH - 1) and uh == 1 and qq == QCH - 1
                        nc.tensor.matmul(
                            scoresT[:, qg:qg + 1],
                            th_t[:, uh, qq],
                            vu_sb[:, uh:uh + 1],
                            start=False, stop=last,
                            skip_group_check=True)

            # ---- exp (already transposed), q-sums via ones matmul, output matmul ----
            et_sb = small.tile([KT, LQ], F32, tag="etsb")
            nc.scalar.activation(et_sb[:], scoresT[:], AF.Exp)
            s_ps = pet.tile([1, LQ], F32, tag="ssum_ps")
            nc.tensor.matmul(s_ps[:], onek_sb[:], et_sb[:], start=True, stop=True)
            s_sb = small.tile([1, LQ], F32, tag="ssum")
            nc.scalar.copy(s_sb[:], s_ps[:])
            o_ps = pout.tile([LQ, DV], F32, tag="opart")
            nc.tensor.matmul(o_ps[:], et_sb[:], vt_sb[:], start=True, stop=True)
            o_sb = small.tile([LQ, DV], F32, tag="osb")
            nc.scalar.copy(o_sb[:], o_ps[:])
            nc.sync.dma_start(o_d[t], o_sb[:])
            nc.sync.dma_start(s_d[t], s_sb[:])

    nc.compile()
    return nc


def kernel(query, key, value, valid_len, W_q, W_k, v):
    query = np.asarray(query, np.float32)
    key = np.asarray(key, np.float32)
    value = np.asarray(value, np.float32)
    vl = np.asarray(valid_len).astype(np.int64)
    W_q = np.asarray(W_q, np.float32)
    W_k = np.asarray(W_k, np.float32)
    v = np.asarray(v, np.float32)

    units = [(b, kt) for b in range(B) for kt in range(-(-int(vl[b]) // KT))]
    T = len(units)
    nt = -(-T // NCORES)
    units += [None] * (NCORES * nt - T)

    if nt not in _cache:
        _cache[nt] = _build(nt)
    nc = _cache[nt]

    wqt = np.ascontiguousarray(W_q.T)                      # [512, 256]
    wkt = np.ascontiguousarray(W_k.T)
    vu = np.ascontiguousarray(v.reshape(2, 128).T)         # [128(ulo), 2(uhi)]
    qts = [np.ascontiguousarray(query[b].T) for b in range(B)]

    in_maps = []
    for c in range(NCORES):
        qt = np.zeros((nt, DQ, LQ), np.float32)
        kt = np.zeros((nt, DK, KT), np.float32)
        vt = np.zeros((nt, KT, DV), np.float32)
        mk = np.full((nt, 1, KT), MASK_VAL, np.float32)
        for j, u in enumerate(units[c * nt:(c + 1) * nt]):
            if u is None:
                continue
            b, k0 = u
            lo, hi = k0 * KT, min((k0 + 1) * KT, LK)
            n = hi - lo
            qt[j] = qts[b]
            kt[j, :, :n] = key[b, lo:hi].T
            vt[j, :n] = value[b, lo:hi]
            valid = min(max(int(vl[b]) - lo, 0), KT)
            mk[j, 0, :valid] = 0.0
        in_maps.append({"qt": qt, "kt": kt, "vt": vt, "mk": mk,
                        "wq": wqt, "wk": wkt, "vu": vu})

    global LAST_RESULT
    LAST_RESULT = run_bass_kernel_spmd(nc, in_maps, list(range(NCORES)), trace=TRACE)
    res = LAST_RESULT.results

    o_sum = np.zeros((B, LQ, DV), np.float64)
    s_sum = np.zeros((B, LQ, 1), np.float64)
    for c in range(NCORES):
        for j, u in enumerate(units[c * nt:(c + 1) * nt]):
            if u is None:
                continue
            b = u[0]
            o_sum[b] += res[c]["o_part"][j]
            s_sum[b] += res[c]["s_part"][j].reshape(LQ, 1)
    return (o_sum / s_sum).astype(np.float32)


# revision 8
# speedup vs baseline: 1.4917x; 1.4917x over previous
"""MLP (additive) attention kernel for Trainium2, 8 NeuronCores.

scores[b,q,k] = sum_u v_u * tanh(qp[b,q,u] + kp[b,k,u]);  qp = query@W_q.T, kp = key@W_k.T
out = softmax(mask(scores)) @ value

Strategy: only k-columns below valid_len[b] are computed (masked columns
contribute exactly 0 to the softmax).  The valid region is split into
128-row k-tiles; each of the 8 cores processes NT = ceil(T/8) independent
(batch, k-tile) units and emits unnormalized partial outputs
O_t = exp(scores_t) @ V_t and row sums s_t; the host sums partials per
batch and divides.  Exact math (no approximations): tanh on ScalarE,
broadcast-add on VectorE/GpSimd, contractions on TensorE, fp32 throughout.
"""

import sys
import numpy as np
from contextlib import ExitStack

sys.path.insert(0, "/opt/trn_rl_repo")

import concourse.bass as bass
import concourse.tile as tile
from concourse import mybir, bacc, masks
from concourse._compat import get_trn_type
from concourse.bass_utils import run_bass_kernel_spmd

F32 = mybir.dt.float32
AF = mybir.ActivationFunctionType

B, LQ, LK = 4, 128, 1024
DQ, DK, DV, U = 512, 512, 512, 256
NCORES = 8
KT = 128            # k-tile rows
QCH = 16            # q rows per add/tanh chunk
MASK_VAL = -1e6

_cache = {}
TRACE = False
LAST_RESULT = None


def _build(nt: int):
    """Build + compile the SPMD program for nt units per core."""
    nc = bacc.Bacc(get_trn_type() or "TRN2", target_bir_lowering=False,
                   debug=False, enable_asserts=True, num_devices=NCORES)

    qt_d = nc.dram_tensor("qt", [nt, DQ, LQ], F32, kind="ExternalInput").ap()
    kt_d = nc.dram_tensor("kt", [nt, DK, KT], F32, kind="ExternalInput").ap()
    vt_d = nc.dram_tensor("vt", [nt, KT, DV], F32, kind="ExternalInput").ap()
    mk_d = nc.dram_tensor("mk", [1, nt * KT], F32, kind="ExternalInput").ap()
    wq_d = nc.dram_tensor("wq", [DQ, U], F32, kind="ExternalInput").ap()
    wk_d = nc.dram_tensor("wk", [DK, U], F32, kind="ExternalInput").ap()
    vu_d = nc.dram_tensor("vu", [128, 2 * 255], F32, kind="ExternalInput").ap()
    o_d = nc.dram_tensor("o_part", [nt, LQ, DV], F32, kind="ExternalOutput").ap()
    s_d = nc.dram_tensor("s_part", [nt, LQ, 1], F32, kind="ExternalOutput").ap()

    with tile.TileContext(nc, trace_sim=False) as tc, ExitStack() as ctx:
        const = ctx.enter_context(tc.tile_pool(name="const", bufs=1))
        io = ctx.enter_context(tc.tile_pool(name="io", bufs=2))
        work = ctx.enter_context(tc.tile_pool(name="work", bufs=2))
        small = ctx.enter_context(tc.tile_pool(name="small", bufs=2))
        pp = ctx.enter_context(tc.tile_pool(name="pp", bufs=2, space="PSUM"))
        psc = ctx.enter_context(tc.tile_pool(name="psc", bufs=2, space="PSUM"))
        pet = ctx.enter_context(tc.tile_pool(name="pet", bufs=1, space="PSUM"))
        pout = ctx.enter_context(tc.tile_pool(name="pout", bufs=2, space="PSUM"))

        # constants
        wq_sb = const.tile([128, 4 * U], F32, tag="wq")   # (dchunk, u)
        wk_sb = const.tile([128, 4 * U], F32, tag="wk")
        vu_sb = const.tile([128, 2 * 255], F32, tag="vu")
        ident = const.tile([128, 128], F32, tag="ident")
        ones_sb = const.tile([1, LQ], F32, tag="ones")
        for d in range(4):
            nc.sync.dma_start(wq_sb[:, d * U:(d + 1) * U], wq_d[bass.ts(d, 128), :])
            nc.sync.dma_start(wk_sb[:, d * U:(d + 1) * U], wk_d[bass.ts(d, 128), :])
        nc.sync.dma_start(vu_sb[:], vu_d[:])
        nc.vector.memset(ones_sb[:], 1.0)
        masks.make_identity(nc, ident[:])

        assert nt % 2 == 0
        for g in range(nt // 2):
            t0 = 2 * g
            projs = []
            vts = []
            for t in (t0, t0 + 1):
                qt_sb = io.tile([128, 4 * LQ], F32, tag="qt")
                kt_sb = io.tile([128, 4 * KT], F32, tag="kt")
                vt_sb = io.tile([KT, DV], F32, tag="vt")
                for d in range(4):
                    nc.sync.dma_start(qt_sb[:, bass.ts(d, LQ)], qt_d[t, bass.ts(d, 128), :])
                    nc.sync.dma_start(kt_sb[:, bass.ts(d, KT)], kt_d[t, bass.ts(d, 128), :])
                nc.sync.dma_start(vt_sb[:], vt_d[t])
                vts.append(vt_sb)

                proj = pp.tile([128, 2 * U], F32, tag="proj")  # 0:256 qp(uh,q), 256:512 kp(uh,k)
                for uh in range(2):
                    for d in range(4):
                        nc.tensor.matmul(
                            proj[:, bass.ts(uh, 128)],
                            wq_sb[:, d * U + uh * 128:d * U + (uh + 1) * 128],
                            qt_sb[:, bass.ts(d, LQ)],
                            start=(d == 0), stop=(d == 3))
                for uh in range(2):
                    for d in range(4):
                        nc.tensor.matmul(
                            proj[:, 256 + uh * 128:256 + (uh + 1) * 128],
                            wk_sb[:, d * U + uh * 128:d * U + (uh + 1) * 128],
                            kt_sb[:, bass.ts(d, KT)],
                            start=(d == 0), stop=(d == 3))
                proj_sb = work.tile([128, 2 * U], F32, tag="projsb")
                nc.vector.tensor_copy(proj_sb[:], proj[:])
                projs.append(proj_sb)

            mk_sb = io.tile([1, 2 * KT], F32, tag="mk")
            nc.sync.dma_start(mk_sb[:], mk_d[:, t0 * KT:(t0 + 2) * KT])

            # scores[q, (unit2, k)] accumulates mask + v-contraction
            scores = psc.tile([LQ, 2 * KT], F32, tag="scores")
            nc.tensor.matmul(scores[:], ones_sb[:], mk_sb[:], start=True, stop=False)

            for qc in range(LQ // QCH):
                # s_t/th_t: [ulo, (uh, q, unit, k)]
                s_t = work.tile([128, 2, QCH, 2, KT], F32, tag="s")
                th_t = work.tile([128, 2, QCH, 2, KT], F32, tag="th")
                for j, proj_sb in enumerate(projs):
                    for uh in range(2):
                        qp_ap = (proj_sb[:, uh * 128 + qc * QCH: uh * 128 + (qc + 1) * QCH]
                                 .unsqueeze(2).broadcast_to([128, QCH, KT]))
                        kp_ap = (proj_sb[:, 256 + uh * 128:256 + (uh + 1) * 128]
                                 .unsqueeze(1).broadcast_to([128, QCH, KT]))
                        eng = nc.vector if (qc + 2 * j + uh) % 3 else nc.gpsimd
                        eng.tensor_add(s_t[:, uh, :, j, :], qp_ap, kp_ap)
                nc.scalar.activation(
                    th_t[:].rearrange("p h q j k -> p (h q j k)"),
                    s_t[:].rearrange("p h q j k -> p (h q j k)"), AF.Tanh)
                # v-dot: shifted-band lhsT (v at column qg), rhs spans both units (N=256)
                for uh in range(2):
                    for qq in range(QCH):
                        qg = qc * QCH + qq
                        last = (qc == LQ // QCH - 1) and uh == 1 and qq == QCH - 1
                        nc.tensor.matmul(
                            scores[:],
                            vu_sb[:, uh * 255 + 127 - qg: uh * 255 + 255 - qg],
                            th_t[:, uh, qq].rearrange("p j k -> p (j k)"),
                            start=False, stop=last,
                            skip_group_check=True)

            for j in range(2):
                e_sb = small.tile([LQ, KT], F32, tag="e")
                s_sb = small.tile([LQ, 1], F32, tag="ssum")
                nc.scalar.activation(e_sb[:], scores[:, bass.ts(j, KT)], AF.Exp,
                                     accum_out=s_sb[:])
                et_ps = pet.tile([KT, LQ], F32, tag="et")
                nc.tensor.transpose(et_ps[:], e_sb[:], ident[:])
                et_sb = small.tile([KT, LQ], F32, tag="etsb")
                nc.vector.tensor_copy(et_sb[:], et_ps[:])
                o_ps = pout.tile([LQ, DV], F32, tag="opart")
                nc.tensor.matmul(o_ps[:], et_sb[:], vts[j][:], start=True, stop=True)
                o_sb = small.tile([LQ, DV], F32, tag="osb")
                nc.scalar.copy(o_sb[:], o_ps[:])
                nc.sync.dma_start(o_d[t0 + j], o_sb[:])
                nc.sync.dma_start(s_d[t0 + j], s_sb[:])

    nc.compile()
    return nc


def kernel(query, key, value, valid_len, W_q, W_k, v):
    query = np.asarray(query, np.float32)
    key = np.asarray(key, np.float32)
    value = np.asarray(value, np.float32)
    vl = np.asarray(valid_len).astype(np.int64)
    W_q = np.asarray(W_q, np.float32)
    W_k = np.asarray(W_k, np.float32)
    v = np.asarray(v, np.float32)

    units = [(b, kt) for b in range(B) for kt in range(-(-int(vl[b]) // KT))]
    T = len(units)
    nt = -(-T // NCORES)
    nt += nt % 2
    units += [None] * (NCORES * nt - T)

    if nt not in _cache:
        _cache[nt] = _build(nt)
    nc = _cache[nt]

    wqt = np.ascontiguousarray(W_q.T)                      # [512, 256]
    wkt = np.ascontiguousarray(W_k.T)
    vu = np.zeros((128, 2, 255), np.float32)               # shifted-column bands
    vu[:, 0, 127] = v[:128]
    vu[:, 1, 127] = v[128:]
    vu = np.ascontiguousarray(vu.reshape(128, 510))
    qts = [np.ascontiguousarray(query[b].T) for b in range(B)]

    in_maps = []
    for c in range(NCORES):
        qt = np.zeros((nt, DQ, LQ), np.float32)
        kt = np.zeros((nt, DK, KT), np.float32)
        vt = np.zeros((nt, KT, DV), np.float32)
        mk = np.full((nt, 1, KT), MASK_VAL, np.float32)  # packed to [1, nt*KT] below
        for j, u in enumerate(units[c * nt:(c + 1) * nt]):
            if u is None:
                continue
            b, k0 = u
            lo, hi = k0 * KT, min((k0 + 1) * KT, LK)
            n = hi - lo
            qt[j] = qts[b]
            kt[j, :, :n] = key[b, lo:hi].T
            vt[j, :n] = value[b, lo:hi]
            valid = min(max(int(vl[b]) - lo, 0), KT)
            mk[j, 0, :valid] = 0.0
        in_maps.append({"qt": qt, "kt": kt, "vt": vt,
                        "mk": np.ascontiguousarray(mk.transpose(1, 0, 2).reshape(1, nt * KT)),
                        "wq": wqt, "wk": wkt, "vu": vu})

    global LAST_RESULT
    LAST_RESULT = run_bass_kernel_spmd(nc, in_maps, list(range(NCORES)), trace=TRACE)
    res = LAST_RESULT.results

    o_sum = np.zeros((B, LQ, DV), np.float64)
    s_sum = np.zeros((B, LQ, 1), np.float64)
    for c in range(NCORES):
        for j, u in enumerate(units[c * nt:(c + 1) * nt]):
            if u is None:
                continue
            b = u[0]
            o_sum[b] += res[c]["o_part"][j]
            s_sum[b] += res[c]["s_part"][j].reshape(LQ, 1)
    return (o_sum / s_sum).astype(np.float32)


# revision 9
# speedup vs baseline: 1.9116x; 1.2815x over previous
"""MLP (additive) attention kernel for Trainium2, 8 NeuronCores.

scores[b,q,k] = sum_u v_u * tanh(qp[b,q,u] + kp[b,k,u]);  qp = query@W_q.T, kp = key@W_k.T
out = softmax(mask(scores)) @ value

Strategy: only k-columns below valid_len[b] are computed (masked columns
contribute exactly 0 to the softmax).  The valid region is split into
128-row k-tiles; each of the 8 cores processes NT = ceil(T/8) independent
(batch, k-tile) units and emits unnormalized partial outputs
O_t = exp(scores_t) @ V_t and row sums s_t; the host sums partials per
batch and divides.  Exact math (no approximations): tanh on ScalarE,
broadcast-add on VectorE/GpSimd, contractions on TensorE, fp32 throughout.
"""

import sys
import numpy as np
from contextlib import ExitStack

sys.path.insert(0, "/opt/trn_rl_repo")

import concourse.bass as bass
import concourse.tile as tile
from concourse import mybir, bacc, masks
from concourse._compat import get_trn_type
from concourse.bass_utils import run_bass_kernel_spmd

F32 = mybir.dt.float32
BF16 = mybir.dt.bfloat16
AF = mybir.ActivationFunctionType

B, LQ, LK = 4, 128, 1024
DQ, DK, DV, U = 512, 512, 512, 256
NCORES = 8
KT = 128            # k-tile rows
QCH = 16            # q rows per add/tanh chunk
MASK_VAL = -1e6

_cache = {}
TRACE = False
LAST_RESULT = None


def _build(nt: int):
    """Build + compile the SPMD program for nt units per core."""
    nc = bacc.Bacc(get_trn_type() or "TRN2", target_bir_lowering=False,
                   debug=False, enable_asserts=True, num_devices=NCORES)

    qt_d = nc.dram_tensor("qt", [nt, DQ, LQ], F32, kind="ExternalInput").ap()
    kt_d = nc.dram_tensor("kt", [nt, DK, KT], F32, kind="ExternalInput").ap()
    vt_d = nc.dram_tensor("vt", [nt, KT, DV], F32, kind="ExternalInput").ap()
    mk_d = nc.dram_tensor("mk", [1, nt * KT], F32, kind="ExternalInput").ap()
    wq_d = nc.dram_tensor("wq", [DQ, U], F32, kind="ExternalInput").ap()
    wk_d = nc.dram_tensor("wk", [DK, U], F32, kind="ExternalInput").ap()
    vu_d = nc.dram_tensor("vu", [128, 2 * 255], BF16, kind="ExternalInput").ap()
    o_d = nc.dram_tensor("o_part", [nt, LQ, DV], F32, kind="ExternalOutput").ap()
    s_d = nc.dram_tensor("s_part", [nt, LQ, 1], F32, kind="ExternalOutput").ap()

    with tile.TileContext(nc, trace_sim=False) as tc, ExitStack() as ctx:
        const = ctx.enter_context(tc.tile_pool(name="const", bufs=1))
        io = ctx.enter_context(tc.tile_pool(name="io", bufs=2))
        work = ctx.enter_context(tc.tile_pool(name="work", bufs=2))
        small = ctx.enter_context(tc.tile_pool(name="small", bufs=2))
        pp = ctx.enter_context(tc.tile_pool(name="pp", bufs=2, space="PSUM"))
        psc = ctx.enter_context(tc.tile_pool(name="psc", bufs=2, space="PSUM"))
        pet = ctx.enter_context(tc.tile_pool(name="pet", bufs=1, space="PSUM"))
        pout = ctx.enter_context(tc.tile_pool(name="pout", bufs=2, space="PSUM"))

        # constants
        wq_sb = const.tile([128, 4 * U], F32, tag="wq")   # (dchunk, u)
        wk_sb = const.tile([128, 4 * U], F32, tag="wk")
        vu_sb = const.tile([128, 2 * 255], BF16, tag="vu")
        ident = const.tile([128, 128], F32, tag="ident")
        ones_sb = const.tile([1, LQ], F32, tag="ones")
        for d in range(4):
            nc.sync.dma_start(wq_sb[:, d * U:(d + 1) * U], wq_d[bass.ts(d, 128), :])
            nc.sync.dma_start(wk_sb[:, d * U:(d + 1) * U], wk_d[bass.ts(d, 128), :])
        nc.sync.dma_start(vu_sb[:], vu_d[:])
        nc.vector.memset(ones_sb[:], 1.0)
        masks.make_identity(nc, ident[:])

        assert nt % 2 == 0
        for g in range(nt // 2):
            t0 = 2 * g
            projs = []
            vts = []
            for t in (t0, t0 + 1):
                qt_sb = io.tile([128, 4 * LQ], F32, tag="qt")
                kt_sb = io.tile([128, 4 * KT], F32, tag="kt")
                vt_sb = io.tile([KT, DV], F32, tag="vt")
                for d in range(4):
                    nc.sync.dma_start(qt_sb[:, bass.ts(d, LQ)], qt_d[t, bass.ts(d, 128), :])
                    nc.sync.dma_start(kt_sb[:, bass.ts(d, KT)], kt_d[t, bass.ts(d, 128), :])
                nc.sync.dma_start(vt_sb[:], vt_d[t])
                vts.append(vt_sb)

                proj = pp.tile([128, 2 * U], F32, tag="proj")  # 0:256 qp(uh,q), 256:512 kp(uh,k)
                for uh in range(2):
                    for d in range(4):
                        nc.tensor.matmul(
                            proj[:, bass.ts(uh, 128)],
                            wq_sb[:, d * U + uh * 128:d * U + (uh + 1) * 128],
                            qt_sb[:, bass.ts(d, LQ)],
                            start=(d == 0), stop=(d == 3))
                for uh in range(2):
                    for d in range(4):
                        nc.tensor.matmul(
                            proj[:, 256 + uh * 128:256 + (uh + 1) * 128],
                            wk_sb[:, d * U + uh * 128:d * U + (uh + 1) * 128],
                            kt_sb[:, bass.ts(d, KT)],
                            start=(d == 0), stop=(d == 3))
                proj_sb = work.tile([128, 2 * U], F32, tag="projsb")
                nc.vector.tensor_copy(proj_sb[:], proj[:])
                projs.append(proj_sb)

            mk_sb = io.tile([1, 2 * KT], F32, tag="mk")
            nc.sync.dma_start(mk_sb[:], mk_d[:, t0 * KT:(t0 + 2) * KT])

            # scores[q, (unit2, k)] accumulates mask + v-contraction
            scores = psc.tile([LQ, 2 * KT], F32, tag="scores")
            nc.tensor.matmul(scores[:], ones_sb[:], mk_sb[:], start=True, stop=False)

            for qc in range(LQ // QCH):
                # s_t/th_t: [ulo, (uh, unit, q, k)] -- contiguous writes per (uh, unit)
                s_t = work.tile([128, 2, 2, QCH, KT], F32, tag="s")
                th_t = work.tile([128, 2, 2, QCH, KT], BF16, tag="th")
                nadd = 0
                for j, proj_sb in enumerate(projs):
                    for uh in range(2):
                        qp_ap = (proj_sb[:, uh * 128 + qc * QCH: uh * 128 + (qc + 1) * QCH]
                                 .unsqueeze(2).broadcast_to([128, QCH, KT]))
                        kp_ap = (proj_sb[:, 256 + uh * 128:256 + (uh + 1) * 128]
                                 .unsqueeze(1).broadcast_to([128, QCH, KT]))
                        idx = 4 * qc + 2 * j + uh
                        eng = nc.vector if (idx * 11) % 16 < 11 else nc.gpsimd
                        eng.tensor_add(s_t[:, uh, j], qp_ap, kp_ap)
                nc.scalar.activation(
                    th_t[:].rearrange("p h j q k -> p (h j q k)"),
                    s_t[:].rearrange("p h j q k -> p (h j q k)"), AF.Tanh)
                # v-dot: shifted-band lhsT (v at column qg), rhs spans both units (N=256)
                for uh in range(2):
                    for qq in range(QCH):
                        qg = qc * QCH + qq
                        last = (qc == LQ // QCH - 1) and uh == 1 and qq == QCH - 1
                        nc.tensor.matmul(
                            scores[:],
                            vu_sb[:, uh * 255 + 127 - qg: uh * 255 + 255 - qg],
                            th_t[:, uh, :, qq, :],
                            start=False, stop=last,
                            skip_group_check=True)

            for j in range(2):
                e_sb = small.tile([LQ, KT], F32, tag="e")
                s_sb = small.tile([LQ, 1], F32, tag="ssum")
                nc.scalar.activation(e_sb[:], scores[:, bass.ts(j, KT)], AF.Exp,
                                     accum_out=s_sb[:])
                et_ps = pet.tile([KT, LQ], F32, tag="et")
                nc.tensor.transpose(et_ps[:], e_sb[:], ident[:])
                et_sb = small.tile([KT, LQ], F32, tag="etsb")
                nc.vector.tensor_copy(et_sb[:], et_ps[:])
                o_ps = pout.tile([LQ, DV], F32, tag="opart")
                nc.tensor.matmul(o_ps[:], et_sb[:], vts[j][:], start=True, stop=True)
                o_sb = small.tile([LQ, DV], F32, tag="osb")
                nc.scalar.copy(o_sb[:], o_ps[:])
                nc.sync.dma_start(o_d[t0 + j], o_sb[:])
                nc.sync.dma_start(s_d[t0 + j], s_sb[:])

    nc.compile()
    return nc


def kernel(query, key, value, valid_len, W_q, W_k, v):
    query = np.asarray(query, np.float32)
    key = np.asarray(key, np.float32)
    value = np.asarray(value, np.float32)
    vl = np.asarray(valid_len).astype(np.int64)
    W_q = np.asarray(W_q, np.float32)
    W_k = np.asarray(W_k, np.float32)
    v = np.asarray(v, np.float32)

    units = [(b, kt) for b in range(B) for kt in range(-(-int(vl[b]) // KT))]
    T = len(units)
    nt = -(-T // NCORES)
    nt += nt % 2
    units += [None] * (NCORES * nt - T)

    if nt not in _cache:
        _cache[nt] = _build(nt)
    nc = _cache[nt]

    wqt = np.ascontiguousarray(W_q.T)                      # [512, 256]
    wkt = np.ascontiguousarray(W_k.T)
    import ml_dtypes
    vu = np.zeros((128, 2, 255), ml_dtypes.bfloat16)       # shifted-column bands
    vu[:, 0, 127] = v[:128]
    vu[:, 1, 127] = v[128:]
    vu = np.ascontiguousarray(vu.reshape(128, 510))
    qts = [np.ascontiguousarray(query[b].T) for b in range(B)]

    in_maps = []
    for c in range(NCORES):
        qt = np.zeros((nt, DQ, LQ), np.float32)
        kt = np.zeros((nt, DK, KT), np.float32)
        vt = np.zeros((nt, KT, DV), np.float32)
        mk = np.full((nt, 1, KT), MASK_VAL, np.float32)  # packed to [1, nt*KT] below
        for j, u in enumerate(units[c * nt:(c + 1) * nt]):
            if u is None:
                continue
            b, k0 = u
            lo, hi = k0 * KT, min((k0 + 1) * KT, LK)
            n = hi - lo
            qt[j] = qts[b]
            kt[j, :, :n] = key[b, lo:hi].T
            vt[j, :n] = value[b, lo:hi]
            valid = min(max(int(vl[b]) - lo, 0), KT)
            mk[j, 0, :valid] = 0.0
        in_maps.append({"qt": qt, "kt": kt, "vt": vt,
                        "mk": np.ascontiguousarray(mk.transpose(1, 0, 2).reshape(1, nt * KT)),
                        "wq": wqt, "wk": wkt, "vu": vu})

    global LAST_RESULT
    LAST_RESULT = run_bass_kernel_spmd(nc, in_maps, list(range(NCORES)), trace=TRACE)
    res = LAST_RESULT.results

    o_sum = np.zeros((B, LQ, DV), np.float64)
    s_sum = np.zeros((B, LQ, 1), np.float64)
    for c in range(NCORES):
        for j, u in enumerate(units[c * nt:(c + 1) * nt]):
            if u is None:
                continue
            b = u[0]
            o_sum[b] += res[c]["o_part"][j]
            s_sum[b] += res[c]["s_part"][j].reshape(LQ, 1)
    return (o_sum / s_sum).astype(np.float32)
